# revision 4
# baseline (speedup 1.0000x reference)
"""Trainium2 Bass kernel for nn_MultiHeadAttention_412316861010.

Sharding: batch x head-group over 8 cores (core c -> batch c//4, heads
(c%4)*4 .. +4). Each core computes its 4 heads' Q/K/V projections (bf16
matmuls), full S x S attention per head (bf16 QK^T, exp on ACT reading
PSUM, bf16 mask multiply on DVE, bf16 PV with a stacked ones-column
producing the softmax denominator in PSUM row 64), softmax normalization,
and a partial output projection (bf16 out). The host sums the 4 partial
projections per batch and adds the output bias.

Fused schedule: q/k/v projections overlap their input DMAs (x_k on the
sync queue, x_q on the DVE queue, x_v on the ACT queue, mask + output on
the GpSimd queue); attention is q-half-outer so the first half's output
projection overlaps the second half's attention.

Self-contained: hardcodes all shapes from the problem spec.
"""
import numpy as np
import ml_dtypes

import concourse.bass as bass
import concourse.mybir as mybir
import concourse.tile as tile
from concourse import bacc
from concourse.bass_utils import run_bass_kernel_spmd

B, S, E, H = 2, 2048, 1024, 16
D = E // H            # 64 head dim
NCORES = 8
HPC = 4               # heads per core
FW = HPC * D          # 256 features per core
F32 = mybir.dt.float32
BF16 = mybir.dt.bfloat16

Exp = mybir.ActivationFunctionType.Exp


def build_nc():
    nc = bacc.Bacc("TRN2", target_bir_lowering=False, debug=False, num_devices=NCORES)

    xqt = nc.dram_tensor("xqt", [E, S], BF16, kind="ExternalInput")
    xkt = nc.dram_tensor("xkt", [E, S], BF16, kind="ExternalInput")
    xvt = nc.dram_tensor("xvt", [E, S], BF16, kind="ExternalInput")
    wq = nc.dram_tensor("wq", [E, FW], BF16, kind="ExternalInput")
    wk = nc.dram_tensor("wk", [E, FW], BF16, kind="ExternalInput")
    wv = nc.dram_tensor("wv", [E, FW], BF16, kind="ExternalInput")
    bq = nc.dram_tensor("bq", [1, FW], BF16, kind="ExternalInput")
    bk = nc.dram_tensor("bk", [1, FW], BF16, kind="ExternalInput")
    bv = nc.dram_tensor("bv", [1, FW], BF16, kind="ExternalInput")
    wo = nc.dram_tensor("wo", [FW, E], BF16, kind="ExternalInput")
    maskt = nc.dram_tensor("maskt", [S, S], BF16, kind="ExternalInput")
    out = nc.dram_tensor("out", [S, E], BF16, kind="ExternalOutput")

    with tile.TileContext(nc) as tc:
        with tc.tile_pool(name="per", bufs=1) as per, \
             tc.tile_pool(name="xp", bufs=3) as xp, \
             tc.tile_pool(name="ep", bufs=4) as ep, \
             tc.tile_pool(name="atp", bufs=6) as atp, \
             tc.tile_pool(name="bcp", bufs=2) as bcp, \
             tc.tile_pool(name="outp", bufs=3) as outp:

            # ---- persistent SBUF ----
            wq_sb = per.tile([128, 8 * FW], BF16, name="wq_sb")
            wk_sb = per.tile([128, 8 * FW], BF16, name="wk_sb")
            wv_sb = per.tile([128, 8 * FW], BF16, name="wv_sb")
            wo_sb = per.tile([128, 2 * E], BF16, name="wo_sb")
            bq_sb = per.tile([1, FW], BF16, name="bq_sb")
            bk_sb = per.tile([1, FW], BF16, name="bk_sb")
            bv_sb = per.tile([1, FW], BF16, name="bv_sb")
            mask_sb = per.tile([128, 16 * S], BF16, name="mask_sb")
            qht_sb = per.tile([128, 2 * S], BF16, name="qht_sb")
            kht_sb = per.tile([128, 2 * S], BF16, name="kht_sb")
            vh_sb = per.tile([128, 16 * 260], BF16, name="vh_sb")
            ctx_sb = per.tile([128, 2 * S], BF16, name="ctx_sb")
            ones_b = per.tile([1, 512], BF16, name="ones_b")
            bv2_sb = per.tile([1, 512], BF16, name="bv2_sb")

            nc.vector.memset(ones_b[:], 1.0)
            nc.vector.memset(vh_sb[:], 1.0)

            # ---- DMAs: weights + x_k on sync, x_q on DVE queue,
            #      x_v on ACT queue, mask on GpSimd queue ----
            for w_sb_, w_ in ((wk_sb, wk), (wq_sb, wq), (wv_sb, wv)):
                nc.sync.dma_start(w_sb_[:].rearrange("p (c n) -> p c n", c=8),
                                  w_.ap().rearrange("(c p) n -> p c n", p=128))
            nc.sync.dma_start(bk_sb[:], bk.ap())
            nc.sync.dma_start(bq_sb[:], bq.ap())
            nc.sync.dma_start(bv_sb[:], bv.ap())
            nc.sync.dma_start(wo_sb[:].rearrange("p (c n) -> p c n", c=2),
                              wo.ap().rearrange("(c p) n -> p c n", p=128))

            xk_t = [xp.tile([128, S], BF16, tag="xk", bufs=3, name=f"x_k{e}")
                    for e in range(8)]
            xq_t = [xp.tile([128, S], BF16, tag="xq", bufs=3, name=f"x_q{e}")
                    for e in range(8)]
            xv_t = [xp.tile([128, S], BF16, tag="xv", bufs=3, name=f"x_v{e}")
                    for e in range(8)]
            for e in range(8):
                nc.sync.dma_start(xk_t[e][:], xkt.ap()[e * 128:(e + 1) * 128, :])
            for e in range(8):
                nc.sync.dma_start(xq_t[e][:], xqt.ap()[e * 128:(e + 1) * 128, :])
            for e in range(8):
                nc.scalar.dma_start(xv_t[e][:], xvt.ap()[e * 128:(e + 1) * 128, :])
            for c in range(16):
                nc.gpsimd.dma_start(mask_sb[:, c * S:(c + 1) * S],
                                    maskt.ap()[c * 128:(c + 1) * 128, :])

            # ================= phase 1: projections =================
            with tc.tile_pool(name="pp", bufs=1, space="PSUM") as pp:
                # k and q projections: khT/qhT [256, S] bf16 as [128, fc*S]
                for nm, xts, w_sb, b_sb, dst in (
                        ("k", xk_t, wk_sb, bk_sb, kht_sb),
                        ("q", xq_t, wq_sb, bq_sb, qht_sb)):
                    accs = [pp.tile([128, 512], F32, tag=f"acc{i}", name=f"acc_{nm}{i}")
                            for i in range(8)]
                    for e in range(8):
                        for fc in range(2):
                            for sq in range(4):
                                nc.tensor.matmul(
                                    accs[fc * 4 + sq][:],
                                    w_sb[:, e * FW + fc * 128: e * FW + fc * 128 + 128],
                                    xts[e][:, sq * 512:(sq + 1) * 512],
                                    start=(e == 0), stop=False)
                    for fc in range(2):
                        for sq in range(4):
                            a = accs[fc * 4 + sq]
                            nc.tensor.matmul(a[:], b_sb[0:1, fc * 128:fc * 128 + 128],
                                             ones_b[0:1, :], start=False, stop=True)
                            nc.vector.tensor_copy(
                                dst[:, fc * S + sq * 512: fc * S + sq * 512 + 512], a[:])

                # v projection: vh [S, 256] -> padded bf16 [128, sk*260] + ones col
                nc.vector.tensor_copy(bv2_sb[0:1, 0:FW], bv_sb[:])
                nc.vector.tensor_copy(bv2_sb[0:1, FW:2 * FW], bv_sb[:])
                accs = [pp.tile([128, 512], F32, tag=f"acc{i}", name=f"acc_v{i}")
                        for i in range(8)]
                for g in range(8):
                    nc.tensor.matmul(accs[g][:], ones_b[0:1, 0:128], bv2_sb[0:1, :],
                                     start=True, stop=False, skip_group_check=True)
                for e in range(8):
                    for g in range(8):
                        for hf in range(2):
                            sk = g * 2 + hf
                            nc.tensor.matmul(
                                accs[g][:, hf * FW:(hf + 1) * FW],
                                xv_t[e][:, sk * 128:(sk + 1) * 128],
                                wv_sb[:, e * FW:(e + 1) * FW],
                                start=False, stop=(e == 7), skip_group_check=True)
                for g in range(8):
                    for hf in range(2):
                        sk = g * 2 + hf
                        nc.vector.tensor_copy(
                            vh_sb[:, sk * 260:(sk + 1) * 260]
                            .rearrange("p (h z) -> p h z", h=4)[:, :, 0:D],
                            accs[g][:, hf * FW:(hf + 1) * FW]
                            .rearrange("p (h z) -> p h z", h=4))

            # ============ phase 2: attention (+ overlapped out-proj) ============
            with tc.tile_pool(name="ap", bufs=1, space="PSUM") as ap:
                for sqh in range(2):
                    q0 = sqh * 1024
                    for h in range(HPC):
                        fc, po = h // 2, (h % 2) * 64
                        ctx_ps = ap.tile([65, 1024], F32, tag="ctx", bufs=1,
                                         name=f"ctx_ps{sqh}_{h}")
                        for sk in range(16):
                            sc_ps = ap.tile([128, 1024], F32, tag="sc", bufs=2,
                                            name=f"sc{sqh}_{h}_{sk}")
                            for i in range(2):
                                nc.tensor.matmul(
                                    sc_ps[:, i * 512:(i + 1) * 512],
                                    kht_sb[po:po + 64,
                                           fc * S + sk * 128: fc * S + sk * 128 + 128],
                                    qht_sb[po:po + 64,
                                           fc * S + q0 + i * 512:
                                           fc * S + q0 + i * 512 + 512],
                                    start=True, stop=True)
                            ex_t = ep.tile([128, 1024], BF16, tag="ex",
                                           name=f"ex{sqh}_{h}_{sk}")
                            nc.scalar.activation(ex_t[:], sc_ps[:], Exp, scale=0.125)
                            at_t = atp.tile([128, 1024], BF16, tag="at",
                                            name=f"at{sqh}_{h}_{sk}")
                            nc.vector.tensor_mul(
                                at_t[:], ex_t[:],
                                mask_sb[:, sk * S + q0: sk * S + q0 + 1024])
                            for i in range(2):
                                nc.tensor.matmul(
                                    ctx_ps[:, i * 512:(i + 1) * 512],
                                    vh_sb[:, sk * 260 + h * 65: sk * 260 + h * 65 + 65],
                                    at_t[:, i * 512:(i + 1) * 512],
                                    start=(sk == 0), stop=(sk == 15))
                        # softmax denominator: row 64 of ctx_ps
                        r_rec = bcp.tile([1, 1024], F32, tag="r_rec",
                                         name=f"r_rec{sqh}_{h}")
                        nc.vector.reciprocal(r_rec[:], ctx_ps[64:65, :])
                        bc_t = bcp.tile([64, 1024], F32, tag="bc",
                                        name=f"bc_t{sqh}_{h}")
                        nc.gpsimd.partition_broadcast(bc_t[:], r_rec[:])
                        nc.vector.tensor_mul(
                            ctx_sb[po:po + 64, fc * S + q0: fc * S + q0 + 1024],
                            ctx_ps[0:64, :], bc_t[:])

                    # ---- out-proj for this q-half ----
                    for qb in range(8):
                        qq = q0 + qb * 128
                        for eh in range(2):
                            op_ps = ap.tile([128, 512], F32, tag="op", bufs=2,
                                            name=f"op{sqh}_{qb}_{eh}")
                            for fcc in range(2):
                                nc.tensor.matmul(
                                    op_ps[:],
                                    ctx_sb[:, fcc * S + qq: fcc * S + qq + 128],
                                    wo_sb[:, fcc * E + eh * 512: fcc * E + eh * 512 + 512],
                                    start=(fcc == 0), stop=(fcc == 1))
                            o_t = outp.tile([128, 512], BF16, tag="o",
                                            name=f"o{sqh}_{qb}_{eh}")
                            nc.vector.tensor_copy(o_t[:], op_ps[:])
                            nc.gpsimd.dma_start(
                                out.ap()[qq:qq + 128, eh * 512:(eh + 1) * 512],
                                o_t[:])

    nc.compile()
    return nc


_CACHE = {}


def _get_nc():
    if "nc" not in _CACHE:
        _CACHE["nc"] = build_nc()
    return _CACHE["nc"]


def make_in_maps(q, k, v, mask, Wqkv, bqkv, Wout):
    bf = ml_dtypes.bfloat16
    maskt = np.ascontiguousarray(mask[0, 0].T).astype(bf)
    xt = [np.ascontiguousarray(np.asarray(a).transpose(0, 2, 1)).astype(bf)
          for a in (q, k, v)]
    Wqkv = np.asarray(Wqkv)
    bqkv = np.asarray(bqkv)
    Wout = np.asarray(Wout)
    in_maps = []
    for c in range(NCORES):
        b = c // 4
        h0 = (c % 4) * HPC
        fsl = slice(h0 * D, (h0 + HPC) * D)
        in_maps.append({
            "xqt": xt[0][b],
            "xkt": xt[1][b],
            "xvt": xt[2][b],
            "wq": np.ascontiguousarray(Wqkv[:, 0:E][:, fsl]).astype(bf),
            "wk": np.ascontiguousarray(Wqkv[:, E:2 * E][:, fsl]).astype(bf),
            "wv": np.ascontiguousarray(Wqkv[:, 2 * E:3 * E][:, fsl]).astype(bf),
            "bq": np.ascontiguousarray(bqkv[0:E][fsl]).reshape(1, FW).astype(bf),
            "bk": np.ascontiguousarray(bqkv[E:2 * E][fsl]).reshape(1, FW).astype(bf),
            "bv": np.ascontiguousarray(bqkv[2 * E:3 * E][fsl]).reshape(1, FW).astype(bf),
            "wo": np.ascontiguousarray(Wout[fsl, :]).astype(bf),
            "maskt": maskt,
        })
    return in_maps


def gather(results, bout):
    out = np.empty((B, S, E), np.float32)
    for b in range(B):
        acc = results[4 * b]["out"].astype(np.float32)
        for c in range(4 * b + 1, 4 * b + 4):
            acc += results[c]["out"].astype(np.float32)
        out[b] = acc + np.asarray(bout)[None, :]
    return out


def kernel(q, k, v, mask, Wqkv, bqkv, Wout, bout):
    nc = _get_nc()
    in_maps = make_in_maps(q, k, v, mask, Wqkv, bqkv, Wout)
    res = run_bass_kernel_spmd(nc, in_maps, core_ids=list(range(NCORES)))
    return gather(res.results, np.asarray(bout))


# revision 5
# speedup vs baseline: 2.4594x; 2.4594x over previous
"""Trainium2 Bass kernel for nn_MultiHeadAttention_412316861010.

Sharding: batch x head-group over 8 cores (core c -> batch c//4, heads
(c%4)*4 .. +4). Each core computes its 4 heads' Q/K/V projections (bf16
matmuls), full S x S attention per head (bf16 QK^T, exp on ACT reading
PSUM, bf16 mask multiply on DVE, bf16 PV with a stacked ones-column
producing the softmax denominator in PSUM row 64), softmax normalization,
and a partial output projection (bf16 out). The host sums the 4 partial
projections per batch and adds the output bias.

Fused schedule: q/k/v projections overlap their input DMAs (x_k on the
sync queue, x_q on the DVE queue, x_v on the ACT queue, mask + output on
the GpSimd queue); attention is q-half-outer so the first half's output
projection overlaps the second half's attention.

Self-contained: hardcodes all shapes from the problem spec.
"""
import numpy as np
import ml_dtypes

import concourse.bass as bass
import concourse.mybir as mybir
import concourse.tile as tile
from concourse import bacc
from concourse.bass_utils import run_bass_kernel_spmd

B, S, E, H = 2, 2048, 1024, 16
D = E // H            # 64 head dim
NCORES = 8
HPC = 4               # heads per core
FW = HPC * D          # 256 features per core
F32 = mybir.dt.float32
BF16 = mybir.dt.bfloat16

Exp = mybir.ActivationFunctionType.Exp


def build_nc():
    nc = bacc.Bacc("TRN2", target_bir_lowering=False, debug=False, num_devices=NCORES)

    xqt = nc.dram_tensor("xqt", [E, S], BF16, kind="ExternalInput")
    xkt = nc.dram_tensor("xkt", [E, S], BF16, kind="ExternalInput")
    xvt = nc.dram_tensor("xvt", [E, S], BF16, kind="ExternalInput")
    wq = nc.dram_tensor("wq", [E, FW], BF16, kind="ExternalInput")
    wk = nc.dram_tensor("wk", [E, FW], BF16, kind="ExternalInput")
    wv = nc.dram_tensor("wv", [E, FW], BF16, kind="ExternalInput")
    bq = nc.dram_tensor("bq", [1, FW], BF16, kind="ExternalInput")
    bk = nc.dram_tensor("bk", [1, FW], BF16, kind="ExternalInput")
    bv = nc.dram_tensor("bv", [1, FW], BF16, kind="ExternalInput")
    wo = nc.dram_tensor("wo", [FW, E], BF16, kind="ExternalInput")
    maskt = nc.dram_tensor("maskt", [S, S], BF16, kind="ExternalInput")
    out = nc.dram_tensor("out", [S, E], BF16, kind="ExternalOutput")

    with tile.TileContext(nc) as tc:
        with tc.tile_pool(name="per", bufs=1) as per, \
             tc.tile_pool(name="xp", bufs=3) as xp, \
             tc.tile_pool(name="ep", bufs=4) as ep, \
             tc.tile_pool(name="atp", bufs=6) as atp, \
             tc.tile_pool(name="bcp", bufs=2) as bcp, \
             tc.tile_pool(name="outp", bufs=3) as outp:

            # ---- persistent SBUF ----
            wq_sb = per.tile([128, 8 * FW], BF16, name="wq_sb")
            wk_sb = per.tile([128, 8 * FW], BF16, name="wk_sb")
            wv_sb = per.tile([128, 8 * FW], BF16, name="wv_sb")
            wo_sb = per.tile([128, 2 * E], BF16, name="wo_sb")
            bq_sb = per.tile([1, FW], BF16, name="bq_sb")
            bk_sb = per.tile([1, FW], BF16, name="bk_sb")
            bv_sb = per.tile([1, FW], BF16, name="bv_sb")
            mask_sb = per.tile([128, 16 * S], BF16, name="mask_sb")
            qht_sb = per.tile([128, 2 * S], BF16, name="qht_sb")
            kht_sb = per.tile([128, 2 * S], BF16, name="kht_sb")
            vh_sb = per.tile([128, 16 * 260], BF16, name="vh_sb")
            ctx_sb = per.tile([128, 2 * S], BF16, name="ctx_sb")
            ones_b = per.tile([1, 512], BF16, name="ones_b")
            bv2_sb = per.tile([1, 512], BF16, name="bv2_sb")

            nc.vector.memset(ones_b[:], 1.0)
            nc.vector.memset(vh_sb[:], 1.0)

            # ---- DMAs: weights + x_k on sync, x_q on DVE queue,
            #      x_v on ACT queue, mask on GpSimd queue ----
            for w_sb_, w_ in ((wk_sb, wk), (wq_sb, wq), (wv_sb, wv)):
                nc.sync.dma_start(w_sb_[:].rearrange("p (c n) -> p c n", c=8),
                                  w_.ap().rearrange("(c p) n -> p c n", p=128))
            nc.sync.dma_start(bk_sb[:], bk.ap())
            nc.sync.dma_start(bq_sb[:], bq.ap())
            nc.sync.dma_start(bv_sb[:], bv.ap())
            nc.sync.dma_start(wo_sb[:].rearrange("p (c n) -> p c n", c=2),
                              wo.ap().rearrange("(c p) n -> p c n", p=128))

            xk_t = [xp.tile([128, S], BF16, tag="xk", bufs=3, name=f"x_k{e}")
                    for e in range(8)]
            xq_t = [xp.tile([128, S], BF16, tag="xq", bufs=3, name=f"x_q{e}")
                    for e in range(8)]
            xv_t = [xp.tile([128, S], BF16, tag="xv", bufs=3, name=f"x_v{e}")
                    for e in range(8)]
            for e in range(8):
                nc.sync.dma_start(xk_t[e][:], xkt.ap()[e * 128:(e + 1) * 128, :])
            for e in range(8):
                nc.sync.dma_start(xq_t[e][:], xqt.ap()[e * 128:(e + 1) * 128, :])
            for e in range(8):
                nc.sync.dma_start(xv_t[e][:], xvt.ap()[e * 128:(e + 1) * 128, :])
            for c in range(16):
                nc.gpsimd.dma_start(mask_sb[:, c * S:(c + 1) * S],
                                    maskt.ap()[c * 128:(c + 1) * 128, :])

            # ================= phase 1: projections =================
            with tc.tile_pool(name="pp", bufs=1, space="PSUM") as pp:
                # k and q projections: khT/qhT [256, S] bf16 as [128, fc*S]
                for nm, xts, w_sb, b_sb, dst in (
                        ("k", xk_t, wk_sb, bk_sb, kht_sb),
                        ("q", xq_t, wq_sb, bq_sb, qht_sb)):
                    accs = [pp.tile([128, 512], F32, tag=f"acc{i}", name=f"acc_{nm}{i}")
                            for i in range(8)]
                    for e in range(8):
                        for fc in range(2):
                            for sq in range(4):
                                nc.tensor.matmul(
                                    accs[fc * 4 + sq][:],
                                    w_sb[:, e * FW + fc * 128: e * FW + fc * 128 + 128],
                                    xts[e][:, sq * 512:(sq + 1) * 512],
                                    start=(e == 0), stop=False)
                    for fc in range(2):
                        for sq in range(4):
                            a = accs[fc * 4 + sq]
                            nc.tensor.matmul(a[:], b_sb[0:1, fc * 128:fc * 128 + 128],
                                             ones_b[0:1, :], start=False, stop=True)
                            nc.vector.tensor_copy(
                                dst[:, fc * S + sq * 512: fc * S + sq * 512 + 512], a[:])

                # v projection: vh [S, 256] -> padded bf16 [128, sk*260] + ones col
                nc.vector.tensor_copy(bv2_sb[0:1, 0:FW], bv_sb[:])
                nc.vector.tensor_copy(bv2_sb[0:1, FW:2 * FW], bv_sb[:])
                accs = [pp.tile([128, 512], F32, tag=f"acc{i}", name=f"acc_v{i}")
                        for i in range(8)]
                for g in range(8):
                    nc.tensor.matmul(accs[g][:], ones_b[0:1, 0:128], bv2_sb[0:1, :],
                                     start=True, stop=False, skip_group_check=True)
                for e in range(8):
                    for g in range(8):
                        for hf in range(2):
                            sk = g * 2 + hf
                            nc.tensor.matmul(
                                accs[g][:, hf * FW:(hf + 1) * FW],
                                xv_t[e][:, sk * 128:(sk + 1) * 128],
                                wv_sb[:, e * FW:(e + 1) * FW],
                                start=False, stop=(e == 7), skip_group_check=True)
                for g in range(8):
                    for hf in range(2):
                        sk = g * 2 + hf
                        nc.vector.tensor_copy(
                            vh_sb[:, sk * 260:(sk + 1) * 260]
                            .rearrange("p (h z) -> p h z", h=4)[:, :, 0:D],
                            accs[g][:, hf * FW:(hf + 1) * FW]
                            .rearrange("p (h z) -> p h z", h=4))

            # ============ phase 2: attention (+ overlapped out-proj) ============
            with tc.tile_pool(name="ap", bufs=1, space="PSUM") as ap:
                for sqh in range(2):
                    q0 = sqh * 1024
                    for h in range(HPC):
                        fc, po = h // 2, (h % 2) * 64
                        ctx_ps = ap.tile([65, 1024], F32, tag="ctx", bufs=1,
                                         name=f"ctx_ps{sqh}_{h}")
                        for sk in range(16):
                            sc_ps = ap.tile([128, 1024], F32, tag="sc", bufs=2,
                                            name=f"sc{sqh}_{h}_{sk}")
                            for i in range(2):
                                nc.tensor.matmul(
                                    sc_ps[:, i * 512:(i + 1) * 512],
                                    kht_sb[po:po + 64,
                                           fc * S + sk * 128: fc * S + sk * 128 + 128],
                                    qht_sb[po:po + 64,
                                           fc * S + q0 + i * 512:
                                           fc * S + q0 + i * 512 + 512],
                                    start=True, stop=True)
                            ex_t = ep.tile([128, 1024], BF16, tag="ex",
                                           name=f"ex{sqh}_{h}_{sk}")
                            nc.scalar.activation(ex_t[:], sc_ps[:], Exp, scale=0.125)
                            at_t = atp.tile([128, 1024], BF16, tag="at",
                                            name=f"at{sqh}_{h}_{sk}")
                            nc.vector.tensor_mul(
                                at_t[:], ex_t[:],
                                mask_sb[:, sk * S + q0: sk * S + q0 + 1024])
                            for i in range(2):
                                nc.tensor.matmul(
                                    ctx_ps[:, i * 512:(i + 1) * 512],
                                    vh_sb[:, sk * 260 + h * 65: sk * 260 + h * 65 + 65],
                                    at_t[:, i * 512:(i + 1) * 512],
                                    start=(sk == 0), stop=(sk == 15))
                        # softmax denominator: row 64 of ctx_ps
                        r_rec = bcp.tile([1, 1024], F32, tag="r_rec",
                                         name=f"r_rec{sqh}_{h}")
                        nc.vector.reciprocal(r_rec[:], ctx_ps[64:65, :])
                        bc_t = bcp.tile([64, 1024], F32, tag="bc",
                                        name=f"bc_t{sqh}_{h}")
                        nc.gpsimd.partition_broadcast(bc_t[:], r_rec[:])
                        nc.vector.tensor_mul(
                            ctx_sb[po:po + 64, fc * S + q0: fc * S + q0 + 1024],
                            ctx_ps[0:64, :], bc_t[:])

                    # ---- out-proj for this q-half ----
                    for qb in range(8):
                        qq = q0 + qb * 128
                        for eh in range(2):
                            op_ps = ap.tile([128, 512], F32, tag="op", bufs=2,
                                            name=f"op{sqh}_{qb}_{eh}")
                            for fcc in range(2):
                                nc.tensor.matmul(
                                    op_ps[:],
                                    ctx_sb[:, fcc * S + qq: fcc * S + qq + 128],
                                    wo_sb[:, fcc * E + eh * 512: fcc * E + eh * 512 + 512],
                                    start=(fcc == 0), stop=(fcc == 1))
                            o_t = outp.tile([128, 512], BF16, tag="o",
                                            name=f"o{sqh}_{qb}_{eh}")
                            nc.vector.tensor_copy(o_t[:], op_ps[:])
                            nc.gpsimd.dma_start(
                                out.ap()[qq:qq + 128, eh * 512:(eh + 1) * 512],
                                o_t[:])

    nc.compile()
    return nc


_CACHE = {}


def _get_nc():
    if "nc" not in _CACHE:
        _CACHE["nc"] = build_nc()
    return _CACHE["nc"]


def make_in_maps(q, k, v, mask, Wqkv, bqkv, Wout):
    bf = ml_dtypes.bfloat16
    maskt = np.ascontiguousarray(mask[0, 0].T).astype(bf)
    xt = [np.ascontiguousarray(np.asarray(a).transpose(0, 2, 1)).astype(bf)
          for a in (q, k, v)]
    Wqkv = np.asarray(Wqkv)
    bqkv = np.asarray(bqkv)
    Wout = np.asarray(Wout)
    in_maps = []
    for c in range(NCORES):
        b = c // 4
        h0 = (c % 4) * HPC
        fsl = slice(h0 * D, (h0 + HPC) * D)
        in_maps.append({
            "xqt": xt[0][b],
            "xkt": xt[1][b],
            "xvt": xt[2][b],
            "wq": np.ascontiguousarray(Wqkv[:, 0:E][:, fsl]).astype(bf),
            "wk": np.ascontiguousarray(Wqkv[:, E:2 * E][:, fsl]).astype(bf),
            "wv": np.ascontiguousarray(Wqkv[:, 2 * E:3 * E][:, fsl]).astype(bf),
            "bq": np.ascontiguousarray(bqkv[0:E][fsl]).reshape(1, FW).astype(bf),
            "bk": np.ascontiguousarray(bqkv[E:2 * E][fsl]).reshape(1, FW).astype(bf),
            "bv": np.ascontiguousarray(bqkv[2 * E:3 * E][fsl]).reshape(1, FW).astype(bf),
            "wo": np.ascontiguousarray(Wout[fsl, :]).astype(bf),
            "maskt": maskt,
        })
    return in_maps


def gather(results, bout):
    out = np.empty((B, S, E), np.float32)
    for b in range(B):
        acc = results[4 * b]["out"].astype(np.float32)
        for c in range(4 * b + 1, 4 * b + 4):
            acc += results[c]["out"].astype(np.float32)
        out[b] = acc + np.asarray(bout)[None, :]
    return out


def kernel(q, k, v, mask, Wqkv, bqkv, Wout, bout):
    nc = _get_nc()
    in_maps = make_in_maps(q, k, v, mask, Wqkv, bqkv, Wout)
    res = run_bass_kernel_spmd(nc, in_maps, core_ids=list(range(NCORES)))
    return gather(res.results, np.asarray(bout))
